# revision 30
# baseline (speedup 1.0000x reference)
"""Trainium2 Bass kernel for the temporal/spatial adapter transformer block.

Sharding: data-parallel over the video batch B=8 -> 1 video (16 frames) per
NeuronCore; all weights replicated. No collectives.

Per-core layout strategy:
  - token-major fp32 residual stream (LayerNorm stats via bn_stats,
    per-partition mean/rstd scalars),
  - feature-major bf16 compute stream for all matmul chains,
  - PE transposes only on bf16 tensors (1 cycle/row),
  - attention computed with transposed scores (S^T = k^T . q) so softmax
    normalization reduces over the partition dim via ones-matmuls; the
    1/sum normalization is deferred onto o via a PE broadcast.
"""

import sys

import numpy as np
import ml_dtypes

try:
    import concourse.bass  # noqa: F401
except ImportError:  # concourse ships with the container, not on sys.path
    for p in ("/opt/trn_rl_repo", "/root/.axon_site/_ro/trn_rl_repo"):
        if p not in sys.path:
            sys.path.insert(0, p)

import concourse.bass as bass
import concourse.mybir as mybir
import concourse.tile as tile
from concourse import bacc
from concourse.bass_utils import run_bass_kernel_spmd

BF = mybir.dt.bfloat16
F32 = mybir.dt.float32
AF = mybir.ActivationFunctionType
OP = mybir.AluOpType

P = 128
NSEQ = 197          # tokens per frame/sequence
D = 768
DK = D // P         # 6
H = 12
HD = 64
BOT = 192
HID = 4 * D         # 3072
HK = HID // P       # 24
EPS = 1e-5
T = 16              # frames per video
TT = 8              # temporal frames
NCORES = 8
TAU = 2 * NSEQ      # tokens per pair = 394
ROWS = T * NSEQ     # 3152 rows per core

GELU_C = 0.044715
GELU_S = 0.7978845608028654  # sqrt(2/pi)
QK_SCALE = HD ** -0.5

ADAPTERS = ("tab", "sa", "ta", "sm", "tm")

bf16 = ml_dtypes.bfloat16


# ----------------------------------------------------------------------------
# host-side weight preprocessing (shared by all cores)
# ----------------------------------------------------------------------------

def preprocess_weights(inp):
    """Build the per-core constant input arrays (already in SBUF layout)."""
    w = {}

    def fm(mat):  # [out, in] -> lhsT layout [128, in//128, out]
        o, i = mat.shape
        return np.ascontiguousarray(
            mat.T.reshape(i // P, P, o).transpose(1, 0, 2)).astype(bf16)

    qkv = np.asarray(inp["qkv_w"], np.float32).copy()
    qkv[:D] *= QK_SCALE  # fold attention scale into q
    w["wqkv"] = fm(qkv)                                   # [128, 6, 2304]

    w["wproj"] = fm(np.asarray(inp["proj_w"], np.float32))  # [128, 6, 768]
    w["bproj"] = np.asarray(inp["proj_b"], np.float32).reshape(DK, P).T.copy()

    a = fm(np.asarray(inp["fc1_w"], np.float32))            # [128, 6, 3072]
    w["wfc1"] = np.ascontiguousarray(
        a.reshape(P, DK, HK, P).transpose(2, 0, 1, 3).reshape(HK, P, DK * P))
    b1 = np.asarray(inp["fc1_b"], np.float32)
    w["bfc1"] = b1.reshape(HK, P).T.copy()                  # [128, 24]
    w["bfc1s"] = (1.702 * b1).reshape(HK, P).T.copy()
    a = fm(np.asarray(inp["fc2_w"], np.float32))            # [128, 24, 768]
    w["wfc2"] = np.ascontiguousarray(
        a.reshape(P, HK, DK, P).transpose(2, 0, 1, 3).reshape(DK, P, HK * P))
    w["bfc2"] = np.asarray(inp["fc2_b"], np.float32).reshape(DK, P).T.copy()

    for ad in ADAPTERS:
        dw = np.asarray(inp[ad + "_dw"], np.float32)        # [192, 768]
        db = np.asarray(inp[ad + "_db"], np.float32)        # [192]
        uw = np.asarray(inp[ad + "_uw"], np.float32)        # [768, 192]
        ub = np.asarray(inp[ad + "_ub"], np.float32)        # [768]
        w["w%sd" % ad] = fm(dw)                             # [128, 6, 192]
        bd = np.zeros((P, 2), np.float32)
        bd[:, 0] = db[:P]
        bd[:64, 1] = db[P:]
        w["b%sd" % ad] = bd
        # up: lhsT [192, 768] -> [128, 2, 768], chunk1 rows 64:128 zero;
        # pre-scaled by 0.5 for the (1+tanh)*u gelu trick
        up = np.zeros((2 * P, D), np.float32)
        up[:BOT] = 0.5 * uw.T
        w["w%su" % ad] = up.reshape(2, P, D).transpose(1, 0, 2).astype(bf16)
        w["b%su" % ad] = ub.reshape(DK, P).T.copy()

    for nm, key in (("g1", "n1_g"), ("b1", "n1_b"), ("g2", "n2_g"), ("b2", "n2_b")):
        w[nm] = np.asarray(inp[key], np.float32).reshape(DK, P).T.copy()

    w["ident"] = np.eye(P, dtype=bf16)
    w["ones"] = np.ones((P, P), dtype=bf16)
    w["epsc"] = np.full((P, 1), EPS, np.float32)
    return w


STREAMED_SPECS = [
    ("wfc1", [HK, P, DK * P], BF),
    ("wfc2", [DK, P, HK * P], BF),
]

WEIGHT_SPECS = [
    ("wqkv", [P, DK, 3 * D], BF),
    ("wproj", [P, DK, D], BF), ("bproj", [P, DK], F32),
    ("bfc1", [P, HK], F32), ("bfc1s", [P, HK], F32),
    ("bfc2", [P, DK], F32),
    ("g1", [P, DK], F32), ("b1", [P, DK], F32),
    ("g2", [P, DK], F32), ("b2", [P, DK], F32),
    ("ident", [P, P], BF), ("ones", [P, P], BF), ("epsc", [P, 1], F32),
] + [
    it for ad in ADAPTERS for it in [
        ("w%sd" % ad, [P, DK, BOT], BF),
        ("b%sd" % ad, [P, 2], F32),
        ("w%su" % ad, [P, 2, D], BF),
        ("b%su" % ad, [P, DK], F32),
    ]
]


# ----------------------------------------------------------------------------
# program emission
# ----------------------------------------------------------------------------

# token tiles of a pair: (row_offset_within_pair, nrows, fm_col_offset)
PAIR_TILES = [(0, P, 0), (P, NSEQ - P, P),
              (NSEQ, P, NSEQ), (NSEQ + P, NSEQ - P, NSEQ + P)]


class Ctx:
    pass


def make_pools(ctx, tc, es):
    def pool(name, bufs):
        return es.enter_context(tc.tile_pool(name=name, bufs=bufs))

    def ppool(name, bufs):
        return es.enter_context(tc.tile_pool(name=name, bufs=bufs, space="PSUM"))

    ctx.weights = pool("weights", 1)
    ctx.xres = pool("xres", 6)       # token-major f32 residual stream
    ctx.small = pool("small", 8)     # bn stats etc.
    ctx.xn = pool("xn", 2)           # token-major bf16 LN output
    ctx.fmA = pool("fmA", 2)         # xnT / xn2T
    ctx.fmB = pool("fmB", 2)         # tab-out / attnT / mlpT (matmul inputs)
    ctx.fmC = pool("fmC", 2)         # delta tiles
    ctx.qk = pool("qk", 2)           # q,k feature-major
    ctx.vt = pool("vt", 3)           # v token-major
    ctx.oT = pool("oT", 2)
    ctx.sa = pool("sa", 2)           # saT / smT
    ctx.ae = pool("ae", 3)           # exp'd scores bf16
    ctx.rr = pool("rr", 3)           # softmax recip (f32+bf16 tiny)
    ctx.rbs = pool("rbs", 1)         # broadcast recip SBUF f32
    ctx.g2 = pool("g2", 1)           # mlp gelu output
    ctx.wf1 = pool("wf1", 3)         # streamed fc1 weight tiles
    ctx.wf2 = pool("wf2", 2)         # streamed fc2 weight tiles
    ctx.sg = pool("sg", 2)           # sigmoid scratch
    ctx.u = pool("u", 2)             # adapter gelu scratch f32

    ctx.pmm = ppool("pmm", 2)        # dense matmul outputs [128, TAU]
    ctx.ptp = ppool("ptp", 2)        # transposes [128, 128]
    ctx.psT = ppool("psT", 1)        # scores
    ctx.prb = ppool("prb", 1)        # softmax sum + broadcast
    ctx.po = ppool("po", 2)          # attention o


def load_weights(ctx, nc, d):
    ctx.W = {}
    for name, shape, dt in WEIGHT_SPECS:
        t = ctx.weights.tile(shape, dt, tag=name)
        nc.sync.dma_start(t[:], d[name][:])
        ctx.W[name] = t


def emit_ln(ctx, nc, xts, tiles, gname, bname):
    """token-major LN on xts (f32) -> feature-major bf16 [128, DK, TAU]."""
    W = ctx.W
    xns = []
    for i, (r0, pi, co) in enumerate(tiles):
        xt = xts[i]
        st = ctx.small.tile([P, 2, 6], F32, tag="bnst")
        nc.vector.bn_stats(st[:pi, 0, :], xt[:pi, 0:D // 2])
        nc.vector.bn_stats(st[:pi, 1, :], xt[:pi, D // 2:D])
        mv = ctx.small.tile([P, 2], F32, tag="bnmv")
        nc.vector.bn_aggr(mv[:pi], st[:pi])
        sd = ctx.small.tile([P, 1], F32, tag="sd")
        nc.scalar.activation(sd[:pi], mv[:pi, 1:2], AF.Sqrt, bias=W["epsc"][:pi, 0:1])
        rstd = ctx.small.tile([P, 1], F32, tag="rstd")
        nc.vector.reciprocal(rstd[:pi], sd[:pi])
        xn = ctx.xn.tile([P, D], BF, tag="xn")
        nc.vector.tensor_scalar(xn[:pi], xt[:pi], mv[:pi, 0:1], rstd[:pi],
                                op0=OP.subtract, op1=OP.mult)
        xns.append(xn)
    xnT = ctx.fmA.tile([P, DK, TAU], BF, tag="xnT")
    for i, (r0, pi, co) in enumerate(tiles):
        for j in range(DK):
            tp = ctx.ptp.tile([P, 1024], BF, tag="tp", name="tp")
            tp = tp[:, :P]
            nc.tensor.transpose(tp[:P, :pi], xns[i][:pi, j * P:(j + 1) * P],
                                W["ident"][:pi, :pi])
            nc.vector.tensor_scalar(xnT[:, j, co:co + pi], tp[:, :pi],
                                    W[gname][:, j:j + 1], W[bname][:, j:j + 1],
                                    op0=OP.mult, op1=OP.add)
    return xnT


def emit_adapter(ctx, nc, ad, inT, combine):
    """adapter ad on feature-major input inT; combine(mc, psum_ap) consumes
    the 6 up-projection psum outputs (bias not yet added)."""
    W = ctx.W
    wd, bd = W["w%sd" % ad], W["b%sd" % ad]
    wu = W["w%su" % ad]
    gs = []
    for oc, (ob, osz) in enumerate(((0, P), (P, 64))):
        ps = ctx.pmm.tile([P, 512], F32, tag="mm", name="mmps")
        ps = ps[:, :TAU]
        for k in range(DK):
            nc.tensor.matmul(ps[:osz], wd[:, k, ob:ob + osz], inT[:, k, :],
                             start=(k == 0), stop=(k == DK - 1))
        u = ctx.u.tile([P, TAU], F32, tag="u")
        nc.scalar.activation(u[:osz], ps[:osz], AF.Identity, bias=bd[:osz, oc:oc + 1])
        u2 = ctx.u.tile([P, TAU], F32, tag="u2")
        nc.scalar.activation(u2[:osz], u[:osz], AF.Square)
        nc.vector.tensor_tensor(u2[:osz], u2[:osz], u[:osz], op=OP.mult)
        nc.vector.scalar_tensor_tensor(u2[:osz], u2[:osz], GELU_C, u[:osz],
                                       op0=OP.mult, op1=OP.add)
        nc.scalar.activation(u2[:osz], u2[:osz], AF.Tanh, scale=GELU_S)
        g = ctx.u.tile([P, TAU], BF, tag="gad%d" % oc)
        nc.vector.scalar_tensor_tensor(g[:osz], u2[:osz], 1.0, u[:osz],
                                       op0=OP.add, op1=OP.mult)
        gs.append(g)
    for mc in range(DK):
        ps = ctx.pmm.tile([P, 512], F32, tag="mm", name="mmps")
        ps = ps[:, :TAU]
        nc.tensor.matmul(ps[:], wu[:, 0, mc * P:(mc + 1) * P], gs[0][:],
                         start=True, stop=False)
        nc.tensor.matmul(ps[:], wu[:64, 1, mc * P:(mc + 1) * P], gs[1][:64],
                         start=False, stop=True)
        combine(mc, ps)


def emit_attention(ctx, nc, inT, tiles):
    """multi-head attention core: feature-major input inT (post-LN/adapter).
    Returns oT (feature-major, softmax-normalized, pre-proj)."""
    W = ctx.W
    wq = ctx.W["wqkv"]
    # q,k feature-major
    qkT = ctx.qk.tile([P, 2 * DK, TAU], BF, tag="qkT")
    for oc in range(2 * DK):
        ps = ctx.pmm.tile([P, 512], F32, tag="mm", name="mmps")
        ps = ps[:, :TAU]
        for k in range(DK):
            nc.tensor.matmul(ps[:], wq[:, k, oc * P:(oc + 1) * P], inT[:, k, :],
                             start=(k == 0), stop=(k == DK - 1))
        nc.scalar.copy(qkT[:, oc, :], ps[:])
    # v token-major
    vts = []
    for i, (r0, pi, co) in enumerate(tiles):
        vt = ctx.vt.tile([P, D], BF, tag="vtok")
        for nb, nsz in ((0, 512), (512, 256)):
            ps = ctx.pmm.tile([P, 512], F32, tag="mm", name="psv")
            for k in range(DK):
                nc.tensor.matmul(ps[:pi, :nsz], inT[:, k, co:co + pi],
                                 wq[:, k, 2 * D + nb:2 * D + nb + nsz],
                                 start=(k == 0), stop=(k == DK - 1))
            nc.any.tensor_copy(vt[:pi, nb:nb + nsz], ps[:pi, :nsz])
        vts.append(vt)
    oT = ctx.oT.tile([P, DK, TAU], BF, tag="oT")
    kts = ((0, P), (P, NSEQ - P))
    for j in range(2):  # seq in pair
        c0 = j * NSEQ
        for h in range(H):
            qof = 64 * (h % 2)
            qch, kch = h // 2, DK + h // 2
            q = qkT[qof:qof + 64, qch, c0:c0 + NSEQ]
            sT = ctx.psT.tile([P, 2, 256], F32, tag="sT", name="sT")
            sT = sT[:, :, :NSEQ]
            for kt, (kb, kp) in enumerate(kts):
                nc.tensor.matmul(sT[:kp, kt, :],
                                 qkT[qof:qof + 64, kch, c0 + kb:c0 + kb + kp],
                                 q, start=True, stop=True)
            ae = ctx.ae.tile([P, 2, NSEQ], BF, tag="ae")
            for kt, (kb, kp) in enumerate(kts):
                nc.scalar.activation(ae[:kp, kt, :], sT[:kp, kt, :], AF.Exp)
            sm = ctx.prb.tile([P, 512], F32, tag="prb", name="sm")
            sm = sm[:, :NSEQ]
            for kt, (kb, kp) in enumerate(kts):
                nc.tensor.matmul(sm[:1, :], W["ones"][:kp, 0:1], ae[:kp, kt, :],
                                 start=(kt == 0), stop=(kt == 1))
            r = ctx.rr.tile([1, NSEQ], F32, tag="r")
            nc.vector.reciprocal(r[:1], sm[:1, :])
            rb = ctx.rr.tile([1, NSEQ], BF, tag="rb")
            nc.vector.tensor_copy(rb[:1], r[:1])
            pb = ctx.prb.tile([P, 512], F32, tag="prb", name="pb")
            pb = pb[:, :NSEQ]
            nc.tensor.matmul(pb[:], W["ones"][0:1, :P], rb[:1], start=True, stop=True)
            rbs = ctx.rbs.tile([P, NSEQ], F32, tag="rbs")
            nc.vector.tensor_copy(rbs[:], pb[:])
            po = ctx.po.tile([P, 512], F32, tag="po", name="po")
            po = po[:, :NSEQ]
            for kt, (kb, kp) in enumerate(kts):
                nc.tensor.matmul(po[qof:qof + 64, :], vts[2 * j + kt][:kp, h * HD:(h + 1) * HD],
                                 ae[:kp, kt, :], start=(kt == 0), stop=(kt == 1))
            nc.vector.tensor_tensor(oT[qof:qof + 64, qch, c0:c0 + NSEQ],
                                    po[qof:qof + 64, :], rbs[qof:qof + 64, :],
                                    op=OP.mult)
    return oT


def emit_matmul_fm(ctx, nc, wname, kn, inT, combine):
    """dense feature-major matmul: out[:, mc, :] for mc in range(6)."""
    w = ctx.W[wname]
    for mc in range(DK):
        ps = ctx.pmm.tile([P, 512], F32, tag="mm", name="mmps")
        ps = ps[:, :TAU]
        for k in range(kn):
            nc.tensor.matmul(ps[:], w[:, k, mc * P:(mc + 1) * P], inT[:, k, :],
                             start=(k == 0), stop=(k == kn - 1))
        combine(mc, ps)


def emit_fc2(ctx, nc, d, g2, combine):
    for mc in range(DK):
        wt = ctx.wf2.tile([P, HK * P], BF, tag="wf2")
        nc.sync.dma_start(wt[:], d["wfc2"][mc])
        ps = ctx.pmm.tile([P, 512], F32, tag="mm", name="mmps")
        ps = ps[:, :TAU]
        for k in range(HK):
            nc.tensor.matmul(ps[:], wt[:, k * P:(k + 1) * P], g2[:, k, :],
                             start=(k == 0), stop=(k == HK - 1))
        combine(mc, ps)


def emit_delta_add(ctx, nc, deltaT, xts, tiles):
    """transpose feature-major delta and accumulate into token-major xts."""
    W = ctx.W
    for i, (r0, pi, co) in enumerate(tiles):
        for j in range(DK):
            tp = ctx.ptp.tile([P, 1024], BF, tag="tp", name="tp")
            tp = tp[:, :P]
            nc.tensor.transpose(tp[:pi, :P], deltaT[:, j, co:co + pi],
                                W["ident"][:, :])
            nc.vector.tensor_tensor(xts[i][:pi, j * P:(j + 1) * P],
                                    xts[i][:pi, j * P:(j + 1) * P],
                                    tp[:pi, :P], op=OP.add)


def emit_pair_gen(ctx, nc, d, branch, rowbase):
    W = ctx.W
    tiles = PAIR_TILES
    # ---- stage A: load + LN1
    xts = []
    for (r0, pi, co) in tiles:
        xt = ctx.xres.tile([P, D], F32, tag="xres")
        nc.sync.dma_start(xt[:pi], d["x"][bass.ds(rowbase + r0, pi), :])
        xts.append(xt)
    xnT = emit_ln(ctx, nc, xts, tiles, "g1", "b1")
    yield

    # ---- branch-specific pre-attention
    if branch == "T":
        aT = ctx.fmB.tile([P, DK, TAU], BF, tag="fmB")

        def tab_comb(mc, ps):
            nc.scalar.activation(aT[:, mc, :], ps[:], AF.Identity,
                                 bias=W["btabu"][:, mc:mc + 1])
        emit_adapter(ctx, nc, "tab", xnT, tab_comb)
        attn_in = aT
        saT = None
    else:
        saT = ctx.sa.tile([P, DK, TAU], BF, tag="saT")

        def sa_comb(mc, ps):
            nc.scalar.activation(saT[:, mc, :], ps[:], AF.Identity,
                                 bias=W["bsau"][:, mc:mc + 1])
        emit_adapter(ctx, nc, "sa", xnT, sa_comb)
        attn_in = xnT
    yield

    # ---- attention
    oT = emit_attention(ctx, nc, attn_in, tiles)
    yield

    # ---- proj (+ branch combine) -> delta1
    delta1 = ctx.fmC.tile([P, DK, TAU], BF, tag="fmC")
    if branch == "T":
        attnT = ctx.fmB.tile([P, DK, TAU], BF, tag="fmB")

        def proj_comb(mc, ps):
            nc.scalar.activation(attnT[:, mc, :], ps[:], AF.Identity,
                                 bias=W["bproj"][:, mc:mc + 1])
        emit_matmul_fm(ctx, nc, "wproj", DK, oT, proj_comb)

        def ta_comb(mc, ps):
            nc.scalar.activation(delta1[:, mc, :], ps[:], AF.Identity,
                                 bias=W["btau"][:, mc:mc + 1])
        emit_adapter(ctx, nc, "ta", attnT, ta_comb)
    else:
        def proj_comb_s(mc, ps):
            nc.vector.scalar_tensor_tensor(delta1[:, mc, :], ps[:],
                                           W["bproj"][:, mc:mc + 1],
                                           saT[:, mc, :],
                                           op0=OP.add, op1=OP.add)
        emit_matmul_fm(ctx, nc, "wproj", DK, oT, proj_comb_s)

    # ---- first residual: x2 = x + delta1 (in-place on xts)
    emit_delta_add(ctx, nc, delta1, xts, tiles)
    yield

    # ---- LN2
    xn2T = emit_ln(ctx, nc, xts, tiles, "g2", "b2")
    yield

    # ---- MLP (+ sm adapter for spatial)
    if branch == "S":
        smT = ctx.sa.tile([P, DK, TAU], BF, tag="saT")

        def sm_comb(mc, ps):
            nc.scalar.activation(smT[:, mc, :], ps[:], AF.Identity,
                                 bias=W["bsmu"][:, mc:mc + 1])
        emit_adapter(ctx, nc, "sm", xn2T, sm_comb)

    g2 = ctx.g2.tile([P, HK, TAU], BF, tag="g2")
    for oc in range(HK):
        wt = ctx.wf1.tile([P, DK * P], BF, tag="wf1")
        nc.sync.dma_start(wt[:], d["wfc1"][oc])
        ps = ctx.pmm.tile([P, 512], F32, tag="mm", name="mmps")
        ps = ps[:, :TAU]
        for k in range(DK):
            nc.tensor.matmul(ps[:], wt[:, k * P:(k + 1) * P],
                             xn2T[:, k, :], start=(k == 0), stop=(k == DK - 1))
        sg = ctx.sg.tile([P, TAU], BF, tag="sg")
        nc.scalar.activation(sg[:], ps[:], AF.Sigmoid, scale=1.702,
                             bias=W["bfc1s"][:, oc:oc + 1])
        nc.vector.scalar_tensor_tensor(g2[:, oc, :], ps[:], W["bfc1"][:, oc:oc + 1],
                                       sg[:], op0=OP.add, op1=OP.mult)
    yield

    delta2 = ctx.fmC.tile([P, DK, TAU], BF, tag="fmC")
    if branch == "T":
        mlpT = ctx.fmB.tile([P, DK, TAU], BF, tag="fmB")

        def fc2_comb(mc, ps):
            nc.scalar.activation(mlpT[:, mc, :], ps[:], AF.Identity,
                                 bias=W["bfc2"][:, mc:mc + 1])
        emit_fc2(ctx, nc, d, g2, fc2_comb)

        def tm_comb(mc, ps):
            nc.scalar.activation(delta2[:, mc, :], ps[:], AF.Identity,
                                 bias=W["btmu"][:, mc:mc + 1])
        emit_adapter(ctx, nc, "tm", mlpT, tm_comb)
    else:
        def fc2_comb_s(mc, ps):
            nc.vector.scalar_tensor_tensor(delta2[:, mc, :], ps[:],
                                           W["bfc2"][:, mc:mc + 1],
                                           smT[:, mc, :], op0=OP.add, op1=OP.add)
        emit_fc2(ctx, nc, d, g2, fc2_comb_s)

    # ---- second residual + store
    emit_delta_add(ctx, nc, delta2, xts, tiles)
    for i, (r0, pi, co) in enumerate(tiles):
        nc.sync.dma_start(d["y"][bass.ds(rowbase + r0, pi), :], xts[i][:pi, :])


def build_program(npairs=4, loop=True, reps=1):
    import contextlib
    nc = bacc.Bacc("TRN2", target_bir_lowering=False, debug=False,
                   num_devices=NCORES)
    d = {}
    d["x"] = nc.dram_tensor("x", [ROWS, D], F32, kind="ExternalInput").ap()
    for name, shape, dt in WEIGHT_SPECS + STREAMED_SPECS:
        d[name] = nc.dram_tensor(name, shape, dt, kind="ExternalInput").ap()
    d["y"] = nc.dram_tensor("y", [ROWS, D], F32, kind="ExternalOutput").ap()

    with tile.TileContext(nc) as tc:
        with contextlib.ExitStack() as es:
            ctx = Ctx()
            make_pools(ctx, tc, es)
            load_weights(ctx, nc, d)

            def body_pairgroup(i):
                for g in (emit_pair_gen(ctx, nc, d, "T", i),
                          emit_pair_gen(ctx, nc, d, "S", i + TT * NSEQ)):
                    for _ in g:
                        pass

            def body_all():
                if loop:
                    with tc.For_i(0, npairs * TAU, TAU, staggered_reset=True) as i:
                        body_pairgroup(i)
                else:
                    for p in range(npairs):
                        body_pairgroup(p * TAU)

            if reps > 1:
                with tc.For_i(0, reps, 1):
                    body_all()
            else:
                body_all()
    nc.compile()
    return nc


# ----------------------------------------------------------------------------
# harness entry point
# ----------------------------------------------------------------------------

_CACHED = {}


def kernel(**inputs):
    if "nc" not in _CACHED:
        _CACHED["nc"] = build_program()
    nc = _CACHED["nc"]
    w = preprocess_weights(inputs)
    x = np.asarray(inputs["x"], np.float32)  # [128, 197, 768]
    in_maps = []
    for c in range(NCORES):
        m = dict(w)
        m["x"] = np.ascontiguousarray(
            x[c * T:(c + 1) * T].reshape(ROWS, D))
        in_maps.append(m)
    res = run_bass_kernel_spmd(nc, in_maps, core_ids=list(range(NCORES)))
    out = np.stack([r["y"].reshape(T, NSEQ, D) for r in res.results])
    return out.reshape(NCORES * T, NSEQ, D)


# revision 32
# speedup vs baseline: 1.0326x; 1.0326x over previous
"""Trainium2 Bass kernel for the temporal/spatial adapter transformer block.

Sharding: data-parallel over the video batch B=8 -> 1 video (16 frames) per
NeuronCore; all weights replicated. No collectives.

Per-core layout strategy:
  - token-major fp32 residual stream (LayerNorm stats via bn_stats,
    per-partition mean/rstd scalars),
  - feature-major bf16 compute stream for all matmul chains,
  - PE transposes only on bf16 tensors (1 cycle/row),
  - attention computed with transposed scores (S^T = k^T . q) so softmax
    normalization reduces over the partition dim via ones-matmuls; the
    1/sum normalization is deferred onto o via a PE broadcast.
"""

import sys

import numpy as np
import ml_dtypes

try:
    import concourse.bass  # noqa: F401
except ImportError:  # concourse ships with the container, not on sys.path
    for p in ("/opt/trn_rl_repo", "/root/.axon_site/_ro/trn_rl_repo"):
        if p not in sys.path:
            sys.path.insert(0, p)

import concourse.bass as bass
import concourse.mybir as mybir
import concourse.tile as tile
from concourse import bacc
from concourse.bass_utils import run_bass_kernel_spmd

BF = mybir.dt.bfloat16
F32 = mybir.dt.float32
AF = mybir.ActivationFunctionType
OP = mybir.AluOpType

P = 128
NSEQ = 197          # tokens per frame/sequence
D = 768
DK = D // P         # 6
H = 12
HD = 64
BOT = 192
HID = 4 * D         # 3072
HK = HID // P       # 24
EPS = 1e-5
T = 16              # frames per video
TT = 8              # temporal frames
NCORES = 8
TAU = 2 * NSEQ      # tokens per pair = 394
ROWS = T * NSEQ     # 3152 rows per core

GELU_C = 0.044715
GELU_S = 0.7978845608028654  # sqrt(2/pi)
QK_SCALE = HD ** -0.5

ADAPTERS = ("tab", "sa", "ta", "sm", "tm")

bf16 = ml_dtypes.bfloat16


# ----------------------------------------------------------------------------
# host-side weight preprocessing (shared by all cores)
# ----------------------------------------------------------------------------

def preprocess_weights(inp):
    """Build the per-core constant input arrays (already in SBUF layout)."""
    w = {}

    def fm(mat):  # [out, in] -> lhsT layout [128, in//128, out]
        o, i = mat.shape
        return np.ascontiguousarray(
            mat.T.reshape(i // P, P, o).transpose(1, 0, 2)).astype(bf16)

    qkv = np.asarray(inp["qkv_w"], np.float32).copy()
    qkv[:D] *= QK_SCALE  # fold attention scale into q
    w["wqkv"] = fm(qkv)                                   # [128, 6, 2304]

    w["wproj"] = fm(np.asarray(inp["proj_w"], np.float32))  # [128, 6, 768]
    w["bproj"] = np.asarray(inp["proj_b"], np.float32).reshape(DK, P).T.copy()

    a = fm(np.asarray(inp["fc1_w"], np.float32))            # [128, 6, 3072]
    w["wfc1"] = np.ascontiguousarray(
        a.reshape(P, DK, HK, P).transpose(2, 0, 1, 3).reshape(HK, P, DK * P))
    b1 = np.asarray(inp["fc1_b"], np.float32)
    w["bfc1"] = b1.reshape(HK, P).T.copy()                  # [128, 24]
    w["bfc1s"] = (1.702 * b1).reshape(HK, P).T.copy()
    a = fm(np.asarray(inp["fc2_w"], np.float32))            # [128, 24, 768]
    w["wfc2"] = np.ascontiguousarray(
        a.reshape(P, HK, DK, P).transpose(2, 0, 1, 3).reshape(DK, P, HK * P))
    w["bfc2"] = np.asarray(inp["fc2_b"], np.float32).reshape(DK, P).T.copy()

    for ad in ADAPTERS:
        dw = np.asarray(inp[ad + "_dw"], np.float32)        # [192, 768]
        db = np.asarray(inp[ad + "_db"], np.float32)        # [192]
        uw = np.asarray(inp[ad + "_uw"], np.float32)        # [768, 192]
        ub = np.asarray(inp[ad + "_ub"], np.float32)        # [768]
        w["w%sd" % ad] = fm(dw)                             # [128, 6, 192]
        bd = np.zeros((P, 2), np.float32)
        bd[:, 0] = db[:P]
        bd[:64, 1] = db[P:]
        w["b%sd" % ad] = bd
        # up: lhsT [192, 768] -> [128, 2, 768], chunk1 rows 64:128 zero;
        # pre-scaled by 0.5 for the (1+tanh)*u gelu trick
        up = np.zeros((2 * P, D), np.float32)
        up[:BOT] = 0.5 * uw.T
        w["w%su" % ad] = up.reshape(2, P, D).transpose(1, 0, 2).astype(bf16)
        w["b%su" % ad] = ub.reshape(DK, P).T.copy()

    for nm, key in (("g1", "n1_g"), ("b1", "n1_b"), ("g2", "n2_g"), ("b2", "n2_b")):
        w[nm] = np.asarray(inp[key], np.float32).reshape(DK, P).T.copy()

    w["ident"] = np.eye(P, dtype=bf16)
    w["ones"] = np.ones((P, P), dtype=bf16)
    w["epsc"] = np.full((P, 1), EPS, np.float32)
    return w


STREAMED_SPECS = [
    ("wfc1", [HK, P, DK * P], BF),
    ("wfc2", [DK, P, HK * P], BF),
]

WEIGHT_SPECS = [
    ("wqkv", [P, DK, 3 * D], BF),
    ("wproj", [P, DK, D], BF), ("bproj", [P, DK], F32),
    ("bfc1", [P, HK], F32), ("bfc1s", [P, HK], F32),
    ("bfc2", [P, DK], F32),
    ("g1", [P, DK], F32), ("b1", [P, DK], F32),
    ("g2", [P, DK], F32), ("b2", [P, DK], F32),
    ("ident", [P, P], BF), ("ones", [P, P], BF), ("epsc", [P, 1], F32),
] + [
    it for ad in ADAPTERS for it in [
        ("w%sd" % ad, [P, DK, BOT], BF),
        ("b%sd" % ad, [P, 2], F32),
        ("w%su" % ad, [P, 2, D], BF),
        ("b%su" % ad, [P, DK], F32),
    ]
]


# ----------------------------------------------------------------------------
# program emission
# ----------------------------------------------------------------------------

# token tiles of a pair: (row_offset_within_pair, nrows, fm_col_offset)
PAIR_TILES = [(0, P, 0), (P, NSEQ - P, P),
              (NSEQ, P, NSEQ), (NSEQ + P, NSEQ - P, NSEQ + P)]


class Ctx:
    pass


def make_pools(ctx, tc, es):
    def pool(name, bufs):
        return es.enter_context(tc.tile_pool(name=name, bufs=bufs))

    def ppool(name, bufs):
        return es.enter_context(tc.tile_pool(name=name, bufs=bufs, space="PSUM"))

    ctx.weights = pool("weights", 1)
    ctx.xres = pool("xres", 6)       # token-major f32 residual stream
    ctx.small = pool("small", 8)     # bn stats etc.
    ctx.xn = pool("xn", 2)           # token-major bf16 LN output
    ctx.fmA = pool("fmA", 2)         # xnT / xn2T
    ctx.fmB = pool("fmB", 2)         # tab-out / attnT / mlpT (matmul inputs)
    ctx.fmC = pool("fmC", 2)         # delta tiles
    ctx.qk = pool("qk", 2)           # q,k feature-major
    ctx.vt = pool("vt", 3)           # v token-major
    ctx.oT = pool("oT", 2)
    ctx.sa = pool("sa", 2)           # saT / smT
    ctx.ae = pool("ae", 3)           # exp'd scores bf16
    ctx.rr = pool("rr", 3)           # softmax recip (f32+bf16 tiny)
    ctx.rbs = pool("rbs", 1)         # broadcast recip SBUF f32
    ctx.g2 = pool("g2", 1)           # mlp gelu output
    ctx.wf1 = pool("wf1", 3)         # streamed fc1 weight tiles
    ctx.wf2 = pool("wf2", 2)         # streamed fc2 weight tiles
    ctx.sg = pool("sg", 2)           # sigmoid scratch
    ctx.u = pool("u", 2)             # adapter gelu scratch f32

    ctx.pmm = ppool("pmm", 2)        # dense matmul outputs [128, TAU]
    ctx.ptp = ppool("ptp", 2)        # transposes [128, 128]
    ctx.psT = ppool("psT", 1)        # scores
    ctx.prb = ppool("prb", 1)        # softmax sum + broadcast
    ctx.po = ppool("po", 2)          # attention o


def load_weights(ctx, nc, d):
    ctx.W = {}
    for name, shape, dt in WEIGHT_SPECS:
        t = ctx.weights.tile(shape, dt, tag=name)
        nc.sync.dma_start(t[:], d[name][:])
        ctx.W[name] = t


def emit_ln(ctx, nc, xts, tiles, gname, bname):
    """token-major LN on xts (f32) -> feature-major bf16 [128, DK, TAU]."""
    W = ctx.W
    xns = []
    for i, (r0, pi, co) in enumerate(tiles):
        xt = xts[i]
        st = ctx.small.tile([P, 2, 6], F32, tag="bnst")
        nc.vector.bn_stats(st[:pi, 0, :], xt[:pi, 0:D // 2])
        nc.vector.bn_stats(st[:pi, 1, :], xt[:pi, D // 2:D])
        mv = ctx.small.tile([P, 2], F32, tag="bnmv")
        nc.vector.bn_aggr(mv[:pi], st[:pi])
        sd = ctx.small.tile([P, 1], F32, tag="sd")
        nc.scalar.activation(sd[:pi], mv[:pi, 1:2], AF.Sqrt, bias=W["epsc"][:pi, 0:1])
        rstd = ctx.small.tile([P, 1], F32, tag="rstd")
        nc.vector.reciprocal(rstd[:pi], sd[:pi])
        xn = ctx.xn.tile([P, D], BF, tag="xn")
        nc.vector.tensor_scalar(xn[:pi], xt[:pi], mv[:pi, 0:1], rstd[:pi],
                                op0=OP.subtract, op1=OP.mult)
        xns.append(xn)
    xnT = ctx.fmA.tile([P, DK, TAU], BF, tag="xnT")
    for i, (r0, pi, co) in enumerate(tiles):
        for j in range(DK):
            tp = ctx.ptp.tile([P, 1024], BF, tag="tp", name="tp")
            tp = tp[:, :P]
            nc.tensor.transpose(tp[:P, :pi], xns[i][:pi, j * P:(j + 1) * P],
                                W["ident"][:pi, :pi])
            nc.vector.tensor_scalar(xnT[:, j, co:co + pi], tp[:, :pi],
                                    W[gname][:, j:j + 1], W[bname][:, j:j + 1],
                                    op0=OP.mult, op1=OP.add)
    return xnT


def emit_adapter(ctx, nc, ad, inT, combine):
    """adapter ad on feature-major input inT; combine(mc, psum_ap) consumes
    the 6 up-projection psum outputs (bias not yet added)."""
    W = ctx.W
    wd, bd = W["w%sd" % ad], W["b%sd" % ad]
    wu = W["w%su" % ad]
    gs = []
    for oc, (ob, osz) in enumerate(((0, P), (P, 64))):
        ps = ctx.pmm.tile([P, 512], F32, tag="mm", name="mmps")
        ps = ps[:, :TAU]
        for k in range(DK):
            nc.tensor.matmul(ps[:osz], wd[:, k, ob:ob + osz], inT[:, k, :],
                             start=(k == 0), stop=(k == DK - 1))
        u = ctx.u.tile([P, TAU], F32, tag="u")
        nc.scalar.activation(u[:osz], ps[:osz], AF.Identity, bias=bd[:osz, oc:oc + 1])
        u2 = ctx.u.tile([P, TAU], F32, tag="u2")
        nc.scalar.activation(u2[:osz], u[:osz], AF.Square)
        nc.vector.tensor_tensor(u2[:osz], u2[:osz], u[:osz], op=OP.mult)
        nc.vector.scalar_tensor_tensor(u2[:osz], u2[:osz], GELU_C, u[:osz],
                                       op0=OP.mult, op1=OP.add)
        nc.scalar.activation(u2[:osz], u2[:osz], AF.Tanh, scale=GELU_S)
        g = ctx.u.tile([P, TAU], BF, tag="gad%d" % oc)
        nc.vector.scalar_tensor_tensor(g[:osz], u2[:osz], 1.0, u[:osz],
                                       op0=OP.add, op1=OP.mult)
        gs.append(g)
    for mc in range(DK):
        ps = ctx.pmm.tile([P, 512], F32, tag="mm", name="mmps")
        ps = ps[:, :TAU]
        nc.tensor.matmul(ps[:], wu[:, 0, mc * P:(mc + 1) * P], gs[0][:],
                         start=True, stop=False)
        nc.tensor.matmul(ps[:], wu[:64, 1, mc * P:(mc + 1) * P], gs[1][:64],
                         start=False, stop=True)
        combine(mc, ps)


def emit_attention(ctx, nc, inT, tiles):
    """multi-head attention core: feature-major input inT (post-LN/adapter).
    Returns oT (feature-major, softmax-normalized, pre-proj)."""
    W = ctx.W
    wq = ctx.W["wqkv"]
    # q,k feature-major
    qkT = ctx.qk.tile([P, 2 * DK, TAU], BF, tag="qkT")
    for oc in range(2 * DK):
        ps = ctx.pmm.tile([P, 512], F32, tag="mm", name="mmps")
        ps = ps[:, :TAU]
        for k in range(DK):
            nc.tensor.matmul(ps[:], wq[:, k, oc * P:(oc + 1) * P], inT[:, k, :],
                             start=(k == 0), stop=(k == DK - 1))
        nc.scalar.copy(qkT[:, oc, :], ps[:])
    # v token-major
    vts = []
    for i, (r0, pi, co) in enumerate(tiles):
        vt = ctx.vt.tile([P, D], BF, tag="vtok")
        for nb, nsz in ((0, 512), (512, 256)):
            ps = ctx.pmm.tile([P, 512], F32, tag="mm", name="psv")
            for k in range(DK):
                nc.tensor.matmul(ps[:pi, :nsz], inT[:, k, co:co + pi],
                                 wq[:, k, 2 * D + nb:2 * D + nb + nsz],
                                 start=(k == 0), stop=(k == DK - 1))
            nc.any.tensor_copy(vt[:pi, nb:nb + nsz], ps[:pi, :nsz])
        vts.append(vt)
    oT = ctx.oT.tile([P, DK, TAU], BF, tag="oT")
    kts = ((0, P), (P, NSEQ - P))
    for j in range(2):  # seq in pair
        c0 = j * NSEQ
        for h in range(H):
            qof = 64 * (h % 2)
            qch, kch = h // 2, DK + h // 2
            q = qkT[qof:qof + 64, qch, c0:c0 + NSEQ]
            sT = ctx.psT.tile([P, 2, 256], F32, tag="sT", name="sT")
            sT = sT[:, :, :NSEQ]
            for kt, (kb, kp) in enumerate(kts):
                nc.tensor.matmul(sT[:kp, kt, :],
                                 qkT[qof:qof + 64, kch, c0 + kb:c0 + kb + kp],
                                 q, start=True, stop=True)
            ae = ctx.ae.tile([P, 2, NSEQ], BF, tag="ae")
            for kt, (kb, kp) in enumerate(kts):
                nc.scalar.activation(ae[:kp, kt, :], sT[:kp, kt, :], AF.Exp)
            sm = ctx.prb.tile([P, 512], F32, tag="prb", name="sm")
            sm = sm[:, :NSEQ]
            for kt, (kb, kp) in enumerate(kts):
                nc.tensor.matmul(sm[:1, :], W["ones"][:kp, 0:1], ae[:kp, kt, :],
                                 start=(kt == 0), stop=(kt == 1))
            r = ctx.rr.tile([1, NSEQ], F32, tag="r")
            nc.vector.reciprocal(r[:1], sm[:1, :])
            rb = ctx.rr.tile([1, NSEQ], BF, tag="rb")
            nc.vector.tensor_copy(rb[:1], r[:1])
            pb = ctx.prb.tile([P, 512], F32, tag="prb", name="pb")
            pb = pb[:, :NSEQ]
            nc.tensor.matmul(pb[:], W["ones"][0:1, :P], rb[:1], start=True, stop=True)
            rbs = ctx.rbs.tile([P, NSEQ], F32, tag="rbs")
            nc.vector.tensor_copy(rbs[:], pb[:])
            po = ctx.po.tile([P, 512], F32, tag="po", name="po")
            po = po[:, :NSEQ]
            for kt, (kb, kp) in enumerate(kts):
                nc.tensor.matmul(po[qof:qof + 64, :], vts[2 * j + kt][:kp, h * HD:(h + 1) * HD],
                                 ae[:kp, kt, :], start=(kt == 0), stop=(kt == 1))
            nc.vector.tensor_tensor(oT[qof:qof + 64, qch, c0:c0 + NSEQ],
                                    po[qof:qof + 64, :], rbs[qof:qof + 64, :],
                                    op=OP.mult)
    return oT


def emit_matmul_fm(ctx, nc, wname, kn, inT, combine):
    """dense feature-major matmul: out[:, mc, :] for mc in range(6)."""
    w = ctx.W[wname]
    for mc in range(DK):
        ps = ctx.pmm.tile([P, 512], F32, tag="mm", name="mmps")
        ps = ps[:, :TAU]
        for k in range(kn):
            nc.tensor.matmul(ps[:], w[:, k, mc * P:(mc + 1) * P], inT[:, k, :],
                             start=(k == 0), stop=(k == kn - 1))
        combine(mc, ps)


def emit_fc2(ctx, nc, d, g2, combine):
    for mc in range(DK):
        wt = ctx.wf2.tile([P, HK * P], BF, tag="wf2")
        nc.sync.dma_start(wt[:], d["wfc2"][mc])
        ps = ctx.pmm.tile([P, 512], F32, tag="mm", name="mmps")
        ps = ps[:, :TAU]
        for k in range(HK):
            nc.tensor.matmul(ps[:], wt[:, k * P:(k + 1) * P], g2[:, k, :],
                             start=(k == 0), stop=(k == HK - 1))
        combine(mc, ps)


def emit_delta_add(ctx, nc, deltaT, xts, tiles):
    """transpose feature-major delta and accumulate into token-major xts."""
    W = ctx.W
    for i, (r0, pi, co) in enumerate(tiles):
        for j in range(DK):
            tp = ctx.ptp.tile([P, 1024], BF, tag="tp", name="tp")
            tp = tp[:, :P]
            nc.tensor.transpose(tp[:pi, :P], deltaT[:, j, co:co + pi],
                                W["ident"][:, :])
            nc.vector.tensor_tensor(xts[i][:pi, j * P:(j + 1) * P],
                                    xts[i][:pi, j * P:(j + 1) * P],
                                    tp[:pi, :P], op=OP.add)


def emit_pair_gen(ctx, nc, d, branch, rowbase):
    W = ctx.W
    tiles = PAIR_TILES
    # ---- stage A: load + LN1
    xts = []
    for (r0, pi, co) in tiles:
        xt = ctx.xres.tile([P, D], F32, tag="xres")
        nc.sync.dma_start(xt[:pi], d["x"][bass.ds(rowbase + r0, pi), :])
        xts.append(xt)
    xnT = emit_ln(ctx, nc, xts, tiles, "g1", "b1")
    yield

    # ---- branch-specific pre-attention
    if branch == "T":
        aT = ctx.fmB.tile([P, DK, TAU], BF, tag="fmB")

        def tab_comb(mc, ps):
            nc.scalar.activation(aT[:, mc, :], ps[:], AF.Identity,
                                 bias=W["btabu"][:, mc:mc + 1])
        emit_adapter(ctx, nc, "tab", xnT, tab_comb)
        attn_in = aT
        saT = None
    else:
        saT = ctx.sa.tile([P, DK, TAU], BF, tag="saT")

        def sa_comb(mc, ps):
            nc.scalar.activation(saT[:, mc, :], ps[:], AF.Identity,
                                 bias=W["bsau"][:, mc:mc + 1])
        emit_adapter(ctx, nc, "sa", xnT, sa_comb)
        attn_in = xnT
    yield

    # ---- attention
    oT = emit_attention(ctx, nc, attn_in, tiles)
    yield

    # ---- proj (+ branch combine) -> delta1
    delta1 = ctx.fmC.tile([P, DK, TAU], BF, tag="fmC")
    if branch == "T":
        attnT = ctx.fmB.tile([P, DK, TAU], BF, tag="fmB")

        def proj_comb(mc, ps):
            nc.scalar.activation(attnT[:, mc, :], ps[:], AF.Identity,
                                 bias=W["bproj"][:, mc:mc + 1])
        emit_matmul_fm(ctx, nc, "wproj", DK, oT, proj_comb)

        def ta_comb(mc, ps):
            nc.scalar.activation(delta1[:, mc, :], ps[:], AF.Identity,
                                 bias=W["btau"][:, mc:mc + 1])
        emit_adapter(ctx, nc, "ta", attnT, ta_comb)
    else:
        def proj_comb_s(mc, ps):
            nc.vector.scalar_tensor_tensor(delta1[:, mc, :], ps[:],
                                           W["bproj"][:, mc:mc + 1],
                                           saT[:, mc, :],
                                           op0=OP.add, op1=OP.add)
        emit_matmul_fm(ctx, nc, "wproj", DK, oT, proj_comb_s)

    # ---- first residual: x2 = x + delta1 (in-place on xts)
    emit_delta_add(ctx, nc, delta1, xts, tiles)
    yield

    # ---- LN2
    xn2T = emit_ln(ctx, nc, xts, tiles, "g2", "b2")
    yield

    # ---- MLP (+ sm adapter for spatial)
    if branch == "S":
        smT = ctx.sa.tile([P, DK, TAU], BF, tag="saT")

        def sm_comb(mc, ps):
            nc.scalar.activation(smT[:, mc, :], ps[:], AF.Identity,
                                 bias=W["bsmu"][:, mc:mc + 1])
        emit_adapter(ctx, nc, "sm", xn2T, sm_comb)

    g2 = ctx.g2.tile([P, HK, TAU], BF, tag="g2")
    for oc in range(HK):
        wt = ctx.wf1.tile([P, DK * P], BF, tag="wf1")
        nc.sync.dma_start(wt[:], d["wfc1"][oc])
        ps = ctx.pmm.tile([P, 512], F32, tag="mm", name="mmps")
        ps = ps[:, :TAU]
        for k in range(DK):
            nc.tensor.matmul(ps[:], wt[:, k * P:(k + 1) * P],
                             xn2T[:, k, :], start=(k == 0), stop=(k == DK - 1))
        sg = ctx.sg.tile([P, TAU], BF, tag="sg")
        nc.scalar.activation(sg[:], ps[:], AF.Sigmoid, scale=1.702,
                             bias=W["bfc1s"][:, oc:oc + 1])
        nc.vector.scalar_tensor_tensor(g2[:, oc, :], ps[:], W["bfc1"][:, oc:oc + 1],
                                       sg[:], op0=OP.add, op1=OP.mult)
    yield

    delta2 = ctx.fmC.tile([P, DK, TAU], BF, tag="fmC")
    if branch == "T":
        mlpT = ctx.fmB.tile([P, DK, TAU], BF, tag="fmB")

        def fc2_comb(mc, ps):
            nc.scalar.activation(mlpT[:, mc, :], ps[:], AF.Identity,
                                 bias=W["bfc2"][:, mc:mc + 1])
        emit_fc2(ctx, nc, d, g2, fc2_comb)

        def tm_comb(mc, ps):
            nc.scalar.activation(delta2[:, mc, :], ps[:], AF.Identity,
                                 bias=W["btmu"][:, mc:mc + 1])
        emit_adapter(ctx, nc, "tm", mlpT, tm_comb)
    else:
        def fc2_comb_s(mc, ps):
            nc.vector.scalar_tensor_tensor(delta2[:, mc, :], ps[:],
                                           W["bfc2"][:, mc:mc + 1],
                                           smT[:, mc, :], op0=OP.add, op1=OP.add)
        emit_fc2(ctx, nc, d, g2, fc2_comb_s)

    # ---- second residual + store
    emit_delta_add(ctx, nc, delta2, xts, tiles)
    for i, (r0, pi, co) in enumerate(tiles):
        nc.sync.dma_start(d["y"][bass.ds(rowbase + r0, pi), :], xts[i][:pi, :])


def build_program(npairs=4, loop=True, reps=1):
    import contextlib
    nc = bacc.Bacc("TRN2", target_bir_lowering=False, debug=False,
                   num_devices=NCORES)
    d = {}
    d["x"] = nc.dram_tensor("x", [ROWS, D], F32, kind="ExternalInput").ap()
    for name, shape, dt in WEIGHT_SPECS + STREAMED_SPECS:
        d[name] = nc.dram_tensor(name, shape, dt, kind="ExternalInput").ap()
    d["y"] = nc.dram_tensor("y", [ROWS, D], F32, kind="ExternalOutput").ap()

    with tile.TileContext(nc) as tc:
        with contextlib.ExitStack() as es:
            ctx = Ctx()
            make_pools(ctx, tc, es)
            load_weights(ctx, nc, d)

            def body_pairgroup(i):
                for g in (emit_pair_gen(ctx, nc, d, "T", i),
                          emit_pair_gen(ctx, nc, d, "S", i + TT * NSEQ)):
                    for _ in g:
                        pass

            def body_all():
                if loop:
                    with tc.For_i(0, npairs * TAU, TAU, staggered_reset=True) as i:
                        body_pairgroup(i)
                else:
                    for p in range(npairs):
                        body_pairgroup(p * TAU)

            if reps > 1:
                with tc.For_i(0, reps, 1):
                    body_all()
            else:
                body_all()
    nc.compile()
    return nc


# ----------------------------------------------------------------------------
# harness entry point
# ----------------------------------------------------------------------------

_CACHED = {}


def kernel(**inputs):
    if "nc" not in _CACHED:
        _CACHED["nc"] = build_program()
    nc = _CACHED["nc"]
    w = preprocess_weights(inputs)
    x = np.asarray(inputs["x"], np.float32)  # [128, 197, 768]
    in_maps = []
    for c in range(NCORES):
        m = dict(w)
        m["x"] = np.ascontiguousarray(
            x[c * T:(c + 1) * T].reshape(ROWS, D))
        in_maps.append(m)
    res = run_bass_kernel_spmd(nc, in_maps, core_ids=list(range(NCORES)))
    out = np.stack([r["y"].reshape(T, NSEQ, D) for r in res.results])
    return out.reshape(NCORES * T, NSEQ, D)
